# revision 4
# baseline (speedup 1.0000x reference)
"""Trainium2 Bass kernel for nn_CenterIdLoss (segment_reduce).

Math restructuring: with S = segment_sum(feat, label) [C, C] and
cnt = bincount(label), every sample with the same label shares a center row,
so the per-sample softmax collapses to a per-class expression:

    loss = (1/(n*m)) * sum_c [ cnt_c * log(ssum_c) - S[c, c] ]
      ssum_c = sum_j exp(S[c, j] / cnt_c)        (cnt clamped to >= 1)

The kernel is ScalarE-bound (exp at 1 elem/lane/cycle @1.2GHz), so the
schedule minimizes ScalarE work: only nonempty classes (~3556 of 4096) are
packed, into 28 global groups of <=128 classes whose sample-row sums are
binpacked to exact multiples of 256 (4 groups @512 rows + 24 @256 for the
reference distribution; a profile-matching greedy lands this exactly).
Each group's 4096 columns split into two 2048-col units -> 56 units spread
7 per core, i.e. 7 ScalarE passes of [128 x 2048] per core instead of the
naive 8. Total DMA bytes are unchanged (each unit ships only its column
half), rows stay balanced, and everything is fp8e4 (|feat| <= ~5.5, final
rel err ~1e-5, far under the 2e-2 gate).

Per stage the segment-sum is DoubleRow fp8 matmuls (256 rows per pass; the
host-shipped one-hot pair block is stationary) accumulated into a
[128, 2048] PSUM half; ScalarE exponentiates straight out of PSUM with the
per-class 1/cnt scale, its free accumulator producing the partial ssum.
Two PSUM halves ping-pong so TensorE and ScalarE overlap. No collectives;
the host does the tiny O(C) epilogue (counts, diagonal, log, final dot) and
sums partials across cores.
"""

import numpy as np
from contextlib import ExitStack

N_TOTAL = 8192
C = 4096
NUM_POS = 4
NCORES = 8
P = 128
HALF = 2048
SCALE = 1.0 / (N_TOTAL * (N_TOTAL // NUM_POS))  # 2^-24

_compile_cache = {}


def _f8np():
    import concourse.mybir as mybir
    return mybir.dt.np(mybir.dt.float8e4)


def _pack_groups(counts):
    """Pack nonempty classes into groups of <=128 classes with row sums at
    multiples of 256. Greedy profile matching: each class goes to the bin
    whose required remaining rows-per-slot best matches its count."""
    ne = np.nonzero(counts)[0]
    c = counts[ne].astype(np.int64)
    order = np.argsort(-c, kind="stable")
    nbig = 4 if len(ne) <= 3584 else 0
    caps = np.array([512] * nbig + [256] * (28 - nbig + max(0, -(-len(ne) // 128) - 28) * 1), np.int64)
    while caps.sum() < c.sum() or len(caps) * 128 < len(ne):
        caps = np.append(caps, 256)
    G = len(caps)
    loads = np.zeros(G, np.int64)
    slots = np.full(G, 128, np.int64)
    grp = np.full(len(ne), -1, np.int64)
    for i in order:
        ci = c[i]
        feas = np.nonzero((slots > 0) & (loads + ci <= caps))[0]
        if len(feas) == 0:
            # bump the cap of some bin with free slots (generic fallback)
            cands = np.nonzero(slots > 0)[0]
            b = cands[np.argmin(loads[cands] + ci - caps[cands])]
            caps[b] = -(-(loads[b] + ci) // 256) * 256
        else:
            req = (caps[feas] - loads[feas]) / slots[feas]
            b = feas[np.argmin(np.abs(ci - req))]
        grp[i] = b
        loads[b] += ci
        slots[b] -= 1
    return ne, grp, caps, loads


def _host_shard_full(feat, label):
    label = np.asarray(label).astype(np.int64)
    feat = np.asarray(feat)
    if feat.dtype != np.float32:
        feat = feat.astype(np.float32)
    counts = np.bincount(label, minlength=C)
    ne, grp, caps, loads = _pack_groups(counts)
    G = len(caps)
    ndbl_g = (caps // 256).astype(np.int64)

    # units: (group, colhalf); shape class = ndbl; pad each shape class to a
    # multiple of NCORES with dummy units so the SPMD stage list is uniform
    units = [(g, h) for g in range(G) for h in (0, 1)]
    by_shape = {}
    for u in units:
        by_shape.setdefault(int(ndbl_g[u[0]]), []).append(u)
    for nd in by_shape:
        while len(by_shape[nd]) % NCORES:
            by_shape[nd].append((-1, 0))  # dummy
    # stage layout: big shapes first
    layout = []
    core_units = [[] for _ in range(NCORES)]
    for nd in sorted(by_shape, reverse=True):
        us = by_shape[nd]
        for s in range(len(us) // NCORES):
            layout.append(nd)
            for cc in range(NCORES):
                core_units[cc].append(us[s * NCORES + cc])
    layout = tuple(layout)
    U = len(layout)
    capR = 256 * sum(layout)

    # per-group class tables: partition index = rank in group
    gcls = [[] for _ in range(G)]
    for i, g in enumerate(grp):
        gcls[g].append(ne[i])
    cnt_g = np.zeros((G, P), np.float32)
    for g in range(G):
        for p, cls in enumerate(gcls[g]):
            cnt_g[g, p] = counts[cls]

    # per-class sample rows (stable order)
    order_n = np.argsort(label, kind="stable")
    starts = np.searchsorted(label[order_n], np.arange(C + 1))

    f8 = _f8np()
    inv_row = (1.0 / np.maximum(counts[label], 1)).astype(np.float32)
    feat8 = (feat * inv_row[:, None]).astype(f8)
    dsum = float(feat.astype(np.float64)[np.arange(len(label)), label].sum())

    # group row index lists (shared by both column halves)
    grow = []
    for g in range(G):
        idx = np.concatenate([order_n[starts[cls]:starts[cls + 1]]
                              for cls in gcls[g]]) if gcls[g] else \
            np.zeros(0, np.int64)
        pvec = np.concatenate([np.full(counts[cls], p, np.int64)
                               for p, cls in enumerate(gcls[g])]) if gcls[g] else \
            np.zeros(0, np.int64)
        grow.append((idx, pvec))

    in_maps = []
    slot_of = {}
    for cc in range(NCORES):
        fused = np.zeros((capR, HALF), f8)
        oh = np.zeros((capR, P), f8)
        r0 = 0
        for s, (g, h) in enumerate(core_units[cc]):
            nd = layout[s]
            if g >= 0:
                idx, pvec = grow[g]
                b = len(idx)
                fused[r0:r0 + b] = feat8[idx, HALF * h:HALF * (h + 1)]
                oh[r0 + np.arange(b), pvec] = 1.0
                slot_of[(g, h)] = (cc, s)
            r0 += 256 * nd
        in_maps.append({"fused": fused, "oh": oh})
    ctx = {"G": G, "cnt_g": cnt_g, "slot_of": slot_of, "dsum": dsum}
    return capR, layout, in_maps, ctx


def _host_shard(feat, label):
    capR, layout, in_maps, _ = _host_shard_full(feat, label)
    return capR, layout, in_maps


def _build(capR, layout, reps=1):
    """Build and compile the SPMD single-core program (same for all cores)."""
    import concourse.tile as tile
    import concourse.mybir as mybir
    from concourse import bacc

    f32 = mybir.dt.float32
    bf16 = mybir.dt.bfloat16
    f8 = mybir.dt.float8e4
    U = len(layout)
    NDC = sum(layout)
    assert capR == 256 * NDC

    nc = bacc.Bacc("TRN2", target_bir_lowering=False, debug=False,
                   num_devices=NCORES)
    fused_d = nc.dram_tensor("fused", [capR, HALF], f8, kind="ExternalInput")
    oh_d = nc.dram_tensor("oh", [capR, P], f8, kind="ExternalInput")
    out_d = nc.dram_tensor("out", [P, U], f32, kind="ExternalOutput")

    with tile.TileContext(nc) as tc, ExitStack() as ctx:
        fp = ctx.enter_context(tc.tile_pool(
            name="fp", bufs=NDC + (2 if reps > 1 else 0)))
        ohp = ctx.enter_context(tc.tile_pool(name="ohp", bufs=2))
        sp = ctx.enter_context(tc.tile_pool(name="sp", bufs=3))
        scr = ctx.enter_context(tc.tile_pool(name="scr", bufs=2))
        pp = ctx.enter_context(tc.tile_pool(name="pp", bufs=2, space="PSUM"))

        def one_pass():
            oh_sb = ohp.tile([P, NDC, 2, P], f8, tag="oh")
            nc.sync.dma_start(
                oh_sb[:], oh_d[:, :].rearrange("(d s p) c -> p d s c", p=P, s=2))
            ssph = sp.tile([P, U], f32, tag="ssph")

            dts = []
            for d in range(NDC):
                t = fp.tile([P, 2, HALF], f8, tag="dc")
                nc.sync.dma_start(
                    t[:],
                    fused_d[256 * d:256 * (d + 1), :]
                    .rearrange("(s p) c -> p s c", p=P))
                dts.append(t)

            d0 = 0
            for u in range(U):
                dlist = list(range(d0, d0 + layout[u]))
                d0 += layout[u]
                pt = pp.tile([P, HALF], f32, tag="pt")
                for j, d in enumerate(dlist):
                    for s in range(HALF // 512):
                        nc.tensor.matmul(
                            pt[:, 512 * s:512 * (s + 1)],
                            oh_sb[:, d, :, :],
                            dts[d][:, :, 512 * s:512 * (s + 1)],
                            start=(j == 0), stop=(j == len(dlist) - 1),
                            perf_mode=mybir.MatmulPerfMode.DoubleRow)
                et = scr.tile([P, HALF], bf16, tag="et")
                nc.scalar.activation(
                    et[:], pt[:], mybir.ActivationFunctionType.Exp,
                    bias=0.0, scale=1.0,
                    accum_out=ssph[:, u:u + 1])
            nc.sync.dma_start(out_d[:, :], ssph[:])

        for _ in range(reps):
            one_pass()

    nc.compile()
    return nc


def _get_program(capR, layout, reps=1):
    key = (capR, tuple(layout), reps)
    if key not in _compile_cache:
        _compile_cache[key] = _build(capR, tuple(layout), reps)
    return _compile_cache[key]


def kernel(**inputs):
    feat = inputs["feat"]
    label = inputs["label"]
    assert feat.shape == (N_TOTAL, C), feat.shape
    capR, layout, in_maps, hctx = _host_shard_full(feat, label)
    nc = _get_program(capR, layout)

    from concourse.bass_utils import run_bass_kernel_spmd
    res = run_bass_kernel_spmd(nc, in_maps, list(range(NCORES)))
    outs = [np.asarray(r["out"], dtype=np.float64) for r in res.results]
    total = 0.0
    for g in range(hctx["G"]):
        c0, s0 = hctx["slot_of"][(g, 0)]
        c1, s1 = hctx["slot_of"][(g, 1)]
        ssum = outs[c0][:, s0] + outs[c1][:, s1]
        total += float((hctx["cnt_g"][g] * np.log(ssum)).sum())
    total -= hctx["dsum"]
    return np.asarray(total * SCALE, dtype=np.float32)
